# revision 2
# baseline (speedup 1.0000x reference)
"""GQA attention with dense RoPE rotation, tensor-parallel over 8 NeuronCores.

Sharding (per spec hint): head-axis tensor parallel. Core i gets q-heads
4i..4i+3 (wq columns), kv-head i (wk/wv columns), and the matching wo rows.
Each core computes a rank-256 partial of out @ wo; the host sums the 8
partials (the all-reduce) and gathers the per-core rotated-k / v shards.

Shapes are hardcoded from the problem spec:
  x [1, 2048, 2048], rotation_matrix [2048, 64, 64], mask [2048, 2048],
  wq [2048, 2048], wk [2048, 512], wv [2048, 512], wo [2048, 2048].
Returns (out [1,2048,2048], k [1,32,2048,64], v [1,32,2048,64]) — matching
the reference, where k is the post-RoPE repeated k and v the repeated v.
"""

import numpy as np

B, L, DIM = 1, 2048, 2048
N_HEADS, N_KV_HEADS, HEAD_DIM = 32, 8, 64
REPEATS = N_HEADS // N_KV_HEADS          # 4
N_CORES = 8
QH = N_HEADS // N_CORES                  # 4 q-heads per core
QCOLS = QH * HEAD_DIM                    # 256 wq columns per core
SCALE = HEAD_DIM ** (-0.5)

_JITTED = None


def _numpy_reference(x, rotation_matrix, mask, wq, wk, wv, wo):
    """Exact fp32 host fallback (faithful port of the reference)."""
    q = (x @ wq).reshape(B, L, N_HEADS, HEAD_DIM).transpose(0, 2, 1, 3)
    k = (x @ wk).reshape(B, L, N_KV_HEADS, HEAD_DIM).transpose(0, 2, 1, 3)
    v = (x @ wv).reshape(B, L, N_KV_HEADS, HEAD_DIM).transpose(0, 2, 1, 3)
    k = np.repeat(k, REPEATS, axis=1)
    v = np.repeat(v, REPEATS, axis=1)
    q = np.einsum('bhlj,lij->bhli', q, rotation_matrix)
    k = np.einsum('bhlj,lij->bhli', k, rotation_matrix)
    scores = np.einsum('bhqd,bhkd->bhqk', q, k) * SCALE
    scores = scores + mask[None, None, :, :]
    m = scores.max(axis=-1, keepdims=True)
    e = np.exp(scores - m)
    probs = e / e.sum(axis=-1, keepdims=True)
    out = np.einsum('bhqk,bhkd->bhqd', probs, v)
    out = out.transpose(0, 2, 1, 3).reshape(B, L, N_HEADS * HEAD_DIM)
    return (out @ wo).astype(np.float32), k.astype(np.float32), v.astype(np.float32)


def _get_jitted():
    """Per-shard program: one compile, run on all 8 cores concurrently."""
    global _JITTED
    if _JITTED is not None:
        return _JITTED
    import jax
    import jax.numpy as jnp

    hp = jax.lax.Precision.HIGHEST

    def shard_fn(x, R, mask, wq_s, wk_s, wv_s, wo_s):
        # x [B,L,D]; wq_s [D,256]; wk_s/wv_s [D,64]; wo_s [256,D]
        q = jnp.matmul(x[0], wq_s, precision=hp)            # [L, 256]
        k = jnp.matmul(x[0], wk_s, precision=hp)            # [L, 64]
        v = jnp.matmul(x[0], wv_s, precision=hp)            # [L, 64]
        q = q.reshape(L, QH, HEAD_DIM).transpose(1, 0, 2)   # [4, L, 64]
        # dense per-position rotation: out[h,l,i] = sum_j q[h,l,j] R[l,i,j].
        # Broadcast-multiply + reduce lowers ~1.5x faster on neuron than the
        # equivalent batched-matmul einsum for these tiny 64x64 contractions.
        qr = (q[:, :, None, :] * R[None]).sum(-1)           # [4, L, 64]
        kr = (k[:, None, :] * R).sum(-1)                    # [L, 64]
        scores = jnp.einsum('hqd,kd->hqk', qr, kr, precision=hp) * SCALE
        scores = scores + mask[None, :, :]
        probs = jax.nn.softmax(scores, axis=-1)
        o = jnp.einsum('hqk,kd->hqd', probs, v, precision=hp)  # [4, L, 64]
        o = o.transpose(1, 0, 2).reshape(L, QCOLS)             # [L, 256]
        partial = jnp.matmul(o, wo_s, precision=hp)            # [L, D]
        return partial, kr, v

    _JITTED = jax.jit(shard_fn)
    return _JITTED


def _kernel_device(x, rotation_matrix, mask, wq, wk, wv, wo):
    import jax

    devs = jax.devices()[:N_CORES]
    fn = _get_jitted()

    futures = []
    for i, d in enumerate(devs):
        args = (
            x,
            rotation_matrix,
            mask,
            wq[:, i * QCOLS:(i + 1) * QCOLS],
            wk[:, i * HEAD_DIM:(i + 1) * HEAD_DIM],
            wv[:, i * HEAD_DIM:(i + 1) * HEAD_DIM],
            wo[i * QCOLS:(i + 1) * QCOLS, :],
        )
        args_d = [jax.device_put(np.ascontiguousarray(a), d) for a in args]
        futures.append(fn(*args_d))          # async dispatch; 8 cores run concurrently

    partials, ks, vs = [], [], []
    for part, kr, vloc in futures:
        partials.append(np.asarray(part))
        ks.append(np.asarray(kr))
        vs.append(np.asarray(vloc))

    out = partials[0]
    for p in partials[1:]:
        out = out + p                        # host all-reduce of the 8 rank-256 partials
    out = out.reshape(B, L, DIM).astype(np.float32)

    k = np.stack(ks, axis=0)[None]           # [1, 8, L, 64]
    v = np.stack(vs, axis=0)[None]
    k = np.repeat(k, REPEATS, axis=1).astype(np.float32)   # [1, 32, L, 64]
    v = np.repeat(v, REPEATS, axis=1).astype(np.float32)
    return out, k, v


def kernel(x, rotation_matrix, mask, wq, wk, wv, wo):
    x = np.asarray(x, dtype=np.float32)
    rotation_matrix = np.asarray(rotation_matrix, dtype=np.float32)
    mask = np.asarray(mask, dtype=np.float32)
    wq = np.asarray(wq, dtype=np.float32)
    wk = np.asarray(wk, dtype=np.float32)
    wv = np.asarray(wv, dtype=np.float32)
    wo = np.asarray(wo, dtype=np.float32)
    try:
        return _kernel_device(x, rotation_matrix, mask, wq, wk, wv, wo)
    except Exception:
        return _numpy_reference(x, rotation_matrix, mask, wq, wk, wv, wo)


# revision 3
# speedup vs baseline: 1.0488x; 1.0488x over previous
"""GQA attention with dense RoPE rotation, tensor-parallel over 8 NeuronCores.

Sharding (per spec hint): head-axis tensor parallel. Core i gets q-heads
4i..4i+3 (wq columns), kv-head i (wk/wv columns), and the matching wo rows.
Each core computes a rank-256 partial of out @ wo; the host sums the 8
partials (the all-reduce) and gathers the per-core rotated-k / v shards.

Shapes are hardcoded from the problem spec:
  x [1, 2048, 2048], rotation_matrix [2048, 64, 64], mask [2048, 2048],
  wq [2048, 2048], wk [2048, 512], wv [2048, 512], wo [2048, 2048].
Returns (out [1,2048,2048], k [1,32,2048,64], v [1,32,2048,64]) — matching
the reference, where k is the post-RoPE repeated k and v the repeated v.
"""

import numpy as np

B, L, DIM = 1, 2048, 2048
N_HEADS, N_KV_HEADS, HEAD_DIM = 32, 8, 64
REPEATS = N_HEADS // N_KV_HEADS          # 4
N_CORES = 8
QH = N_HEADS // N_CORES                  # 4 q-heads per core
QCOLS = QH * HEAD_DIM                    # 256 wq columns per core
SCALE = HEAD_DIM ** (-0.5)

_JITTED = None


def _numpy_reference(x, rotation_matrix, mask, wq, wk, wv, wo):
    """Exact fp32 host fallback (faithful port of the reference)."""
    q = (x @ wq).reshape(B, L, N_HEADS, HEAD_DIM).transpose(0, 2, 1, 3)
    k = (x @ wk).reshape(B, L, N_KV_HEADS, HEAD_DIM).transpose(0, 2, 1, 3)
    v = (x @ wv).reshape(B, L, N_KV_HEADS, HEAD_DIM).transpose(0, 2, 1, 3)
    k = np.repeat(k, REPEATS, axis=1)
    v = np.repeat(v, REPEATS, axis=1)
    q = np.einsum('bhlj,lij->bhli', q, rotation_matrix)
    k = np.einsum('bhlj,lij->bhli', k, rotation_matrix)
    scores = np.einsum('bhqd,bhkd->bhqk', q, k) * SCALE
    scores = scores + mask[None, None, :, :]
    m = scores.max(axis=-1, keepdims=True)
    e = np.exp(scores - m)
    probs = e / e.sum(axis=-1, keepdims=True)
    out = np.einsum('bhqk,bhkd->bhqd', probs, v)
    out = out.transpose(0, 2, 1, 3).reshape(B, L, N_HEADS * HEAD_DIM)
    return (out @ wo).astype(np.float32), k.astype(np.float32), v.astype(np.float32)


def _get_jitted():
    """Per-shard program: one compile, run on all 8 cores concurrently."""
    global _JITTED
    if _JITTED is not None:
        return _JITTED
    import jax
    import jax.numpy as jnp

    hp = jax.lax.Precision.HIGHEST

    def shard_fn(x, R, mask, wq_s, wk_s, wv_s, wo_s):
        # x [B,L,D]; wq_s [D,256]; wk_s/wv_s [D,64]; wo_s [256,D]
        q = jnp.matmul(x[0], wq_s, precision=hp)            # [L, 256]
        k = jnp.matmul(x[0], wk_s, precision=hp)            # [L, 64]
        v = jnp.matmul(x[0], wv_s, precision=hp)            # [L, 64]
        q = q.reshape(L, QH, HEAD_DIM).transpose(1, 0, 2)   # [4, L, 64]
        # dense per-position rotation: out[h,l,i] = sum_j q[h,l,j] R[l,i,j]
        qr = jnp.einsum('hlj,lij->hli', q, R, precision=hp)  # [4, L, 64]
        kr = jnp.einsum('lj,lij->li', k, R, precision=hp)    # [L, 64]
        scores = jnp.einsum('hqd,kd->hqk', qr, kr, precision=hp) * SCALE
        scores = scores + mask[None, :, :]
        probs = jax.nn.softmax(scores, axis=-1)
        o = jnp.einsum('hqk,kd->hqd', probs, v, precision=hp)  # [4, L, 64]
        o = o.transpose(1, 0, 2).reshape(L, QCOLS)             # [L, 256]
        partial = jnp.matmul(o, wo_s, precision=hp)            # [L, D]
        return partial, kr, v

    _JITTED = jax.jit(shard_fn)
    return _JITTED


def _kernel_device(x, rotation_matrix, mask, wq, wk, wv, wo):
    import jax

    devs = jax.devices()[:N_CORES]
    fn = _get_jitted()

    futures = []
    for i, d in enumerate(devs):
        args = (
            x,
            rotation_matrix,
            mask,
            wq[:, i * QCOLS:(i + 1) * QCOLS],
            wk[:, i * HEAD_DIM:(i + 1) * HEAD_DIM],
            wv[:, i * HEAD_DIM:(i + 1) * HEAD_DIM],
            wo[i * QCOLS:(i + 1) * QCOLS, :],
        )
        args_d = [jax.device_put(np.ascontiguousarray(a), d) for a in args]
        futures.append(fn(*args_d))          # async dispatch; 8 cores run concurrently

    partials, ks, vs = [], [], []
    for part, kr, vloc in futures:
        partials.append(np.asarray(part))
        ks.append(np.asarray(kr))
        vs.append(np.asarray(vloc))

    out = partials[0]
    for p in partials[1:]:
        out = out + p                        # host all-reduce of the 8 rank-256 partials
    out = out.reshape(B, L, DIM).astype(np.float32)

    k = np.stack(ks, axis=0)[None]           # [1, 8, L, 64]
    v = np.stack(vs, axis=0)[None]
    k = np.repeat(k, REPEATS, axis=1).astype(np.float32)   # [1, 32, L, 64]
    v = np.repeat(v, REPEATS, axis=1).astype(np.float32)
    return out, k, v


def kernel(x, rotation_matrix, mask, wq, wk, wv, wo):
    x = np.asarray(x, dtype=np.float32)
    rotation_matrix = np.asarray(rotation_matrix, dtype=np.float32)
    mask = np.asarray(mask, dtype=np.float32)
    wq = np.asarray(wq, dtype=np.float32)
    wk = np.asarray(wk, dtype=np.float32)
    wv = np.asarray(wv, dtype=np.float32)
    wo = np.asarray(wo, dtype=np.float32)
    try:
        return _kernel_device(x, rotation_matrix, mask, wq, wk, wv, wo)
    except Exception:
        return _numpy_reference(x, rotation_matrix, mask, wq, wk, wv, wo)


# revision 4
# speedup vs baseline: 1.6239x; 1.5484x over previous
"""GQA attention with dense RoPE rotation, tensor-parallel over 8 NeuronCores.

Sharding (per spec hint): head-axis tensor parallel. Core i gets q-heads
4i..4i+3 (wq columns), kv-head i (wk/wv columns), and the matching wo rows.
Each core computes a rank-256 partial of out @ wo; the host sums the 8
partials (the all-reduce) and gathers the per-core rotated-k / v shards.

Shapes are hardcoded from the problem spec:
  x [1, 2048, 2048], rotation_matrix [2048, 64, 64], mask [2048, 2048],
  wq [2048, 2048], wk [2048, 512], wv [2048, 512], wo [2048, 2048].
Returns (out [1,2048,2048], k [1,32,2048,64], v [1,32,2048,64]) — matching
the reference, where k is the post-RoPE repeated k and v the repeated v.
"""

import numpy as np

B, L, DIM = 1, 2048, 2048
N_HEADS, N_KV_HEADS, HEAD_DIM = 32, 8, 64
REPEATS = N_HEADS // N_KV_HEADS          # 4
N_CORES = 8
QH = N_HEADS // N_CORES                  # 4 q-heads per core
QCOLS = QH * HEAD_DIM                    # 256 wq columns per core
SCALE = HEAD_DIM ** (-0.5)

_JITTED = None


def _numpy_reference(x, rotation_matrix, mask, wq, wk, wv, wo):
    """Exact fp32 host fallback (faithful port of the reference)."""
    q = (x @ wq).reshape(B, L, N_HEADS, HEAD_DIM).transpose(0, 2, 1, 3)
    k = (x @ wk).reshape(B, L, N_KV_HEADS, HEAD_DIM).transpose(0, 2, 1, 3)
    v = (x @ wv).reshape(B, L, N_KV_HEADS, HEAD_DIM).transpose(0, 2, 1, 3)
    k = np.repeat(k, REPEATS, axis=1)
    v = np.repeat(v, REPEATS, axis=1)
    q = np.einsum('bhlj,lij->bhli', q, rotation_matrix)
    k = np.einsum('bhlj,lij->bhli', k, rotation_matrix)
    scores = np.einsum('bhqd,bhkd->bhqk', q, k) * SCALE
    scores = scores + mask[None, None, :, :]
    m = scores.max(axis=-1, keepdims=True)
    e = np.exp(scores - m)
    probs = e / e.sum(axis=-1, keepdims=True)
    out = np.einsum('bhqk,bhkd->bhqd', probs, v)
    out = out.transpose(0, 2, 1, 3).reshape(B, L, N_HEADS * HEAD_DIM)
    return (out @ wo).astype(np.float32), k.astype(np.float32), v.astype(np.float32)


def _get_jitted():
    """Per-shard program: one compile, run on all 8 cores concurrently."""
    global _JITTED
    if _JITTED is not None:
        return _JITTED
    import jax
    import jax.numpy as jnp

    try:
        jax.config.update("jax_compilation_cache_dir", "/tmp/jax_neuron_cache")
        jax.config.update("jax_persistent_cache_min_compile_time_secs", 1.0)
    except Exception:
        pass

    hp = jax.lax.Precision.HIGHEST

    def shard_fn(x, R, mask, wq_s, wk_s, wv_s, wo_s):
        # x [B,L,D]; wq_s [D,256]; wk_s/wv_s [D,64]; wo_s [256,D]
        q = jnp.matmul(x[0], wq_s, precision=hp)            # [L, 256]
        k = jnp.matmul(x[0], wk_s, precision=hp)            # [L, 64]
        v = jnp.matmul(x[0], wv_s, precision=hp)            # [L, 64]
        q = q.reshape(L, QH, HEAD_DIM).transpose(1, 0, 2)   # [4, L, 64]
        # dense per-position rotation: out[h,l,i] = sum_j q[h,l,j] R[l,i,j]
        qr = jnp.einsum('hlj,lij->hli', q, R, precision=hp)  # [4, L, 64]
        kr = jnp.einsum('lj,lij->li', k, R, precision=hp)    # [L, 64]
        scores = jnp.einsum('hqd,kd->hqk', qr, kr, precision=hp) * SCALE
        scores = scores + mask[None, :, :]
        probs = jax.nn.softmax(scores, axis=-1)
        o = jnp.einsum('hqk,kd->hqd', probs, v, precision=hp)  # [4, L, 64]
        o = o.transpose(1, 0, 2).reshape(L, QCOLS)             # [L, 256]
        partial = jnp.matmul(o, wo_s, precision=hp)            # [L, D]
        return partial, kr, v

    _JITTED = jax.jit(shard_fn)
    return _JITTED


def _kernel_device(x, rotation_matrix, mask, wq, wk, wv, wo):
    import jax

    devs = jax.devices()[:N_CORES]
    fn = _get_jitted()

    futures = []
    for i, d in enumerate(devs):
        args = (
            x,
            rotation_matrix,
            mask,
            wq[:, i * QCOLS:(i + 1) * QCOLS],
            wk[:, i * HEAD_DIM:(i + 1) * HEAD_DIM],
            wv[:, i * HEAD_DIM:(i + 1) * HEAD_DIM],
            wo[i * QCOLS:(i + 1) * QCOLS, :],
        )
        args_d = [jax.device_put(np.ascontiguousarray(a), d) for a in args]
        futures.append(fn(*args_d))          # async dispatch; 8 cores run concurrently

    partials, ks, vs = [], [], []
    for part, kr, vloc in futures:
        partials.append(np.asarray(part))
        ks.append(np.asarray(kr))
        vs.append(np.asarray(vloc))

    out = partials[0]
    for p in partials[1:]:
        out = out + p                        # host all-reduce of the 8 rank-256 partials
    out = out.reshape(B, L, DIM).astype(np.float32)

    k = np.stack(ks, axis=0)[None]           # [1, 8, L, 64]
    v = np.stack(vs, axis=0)[None]
    k = np.repeat(k, REPEATS, axis=1).astype(np.float32)   # [1, 32, L, 64]
    v = np.repeat(v, REPEATS, axis=1).astype(np.float32)
    return out, k, v


def kernel(x, rotation_matrix, mask, wq, wk, wv, wo):
    x = np.asarray(x, dtype=np.float32)
    rotation_matrix = np.asarray(rotation_matrix, dtype=np.float32)
    mask = np.asarray(mask, dtype=np.float32)
    wq = np.asarray(wq, dtype=np.float32)
    wk = np.asarray(wk, dtype=np.float32)
    wv = np.asarray(wv, dtype=np.float32)
    wo = np.asarray(wo, dtype=np.float32)
    try:
        return _kernel_device(x, rotation_matrix, mask, wq, wk, wv, wo)
    except Exception:
        return _numpy_reference(x, rotation_matrix, mask, wq, wk, wv, wo)
